# revision 30
# baseline (speedup 1.0000x reference)
"""Trainium2 Bass kernel for nn_Attention_54262616817926.

kernel(x, w_qkv, b_qkv, w_proj, b_proj) -> out [8, 4, 1024, 192] float32.

Sharding: pure data-parallel over batch B=8 across the 8 NeuronCores
(core c computes batch c end-to-end; no collectives). Inputs are
preprocessed host-side (transposed/augmented layouts).

v4 dataflow, tuned against HW microbenchmarks:
  - Every matmul uses a FULL [128, 128] stationary operand: partial
    stationaries (48-row / 64-col) measure ~2.4x slower per 512-col
    stream on TRN2 (fast-weight-load doesn't engage). Per-head
    zero-padded copies of Q / V-blocks / proj weights make the
    cross-head products vanish exactly while keeping the array full.
  - ACT (exp) is then the bottleneck (~1.23us per [128,1024] exp from
    PSUM): all other work is interleaved into the attention kt-loop's
    program order (qkv of pair p+1, proj of pair p-1, V tiles of pair
    p) so the other engines run in ACT's shadow.
  - Softmax normalization happens inside the output projection: the
    proj output is token-major, so 1/l is a per-partition scalar
    multiply fused into the head accumulation (STT ops on DVE). The l
    rows ride along as a ones-column in the V blocks (partitions 0/64
    of the PV accumulator).
  - One-time zero-init of the padded SBUF regions sits OUTSIDE the
    repeat loop (it is loop-invariant state).

Self-contained: inlines the TileContext tail-drain workaround and the
kernel builder; hardcodes B=8, P=4, N=1024, d=192, H=4.
"""

"""Workaround for walrus 'Too many sync wait commands' on the TileContext
tail drain: this build's walrus accepts at most 1 sync wait on a TPB_CTRL
(Drain) instruction, but TileContext._drain_and_barrier packs every
outstanding semaphore wait onto one drain. Split them into one wait-carrying
nop per semaphore, then emit a clean drain."""

import bass_rust
import concourse.mybir as mybir
import concourse.tile as tile
from concourse.vector_clock import ScopedClock

_WAIT_OP = {
    "ge": "sem-ge",
    "sem-ge": "sem-ge",
}


def _patched_drain_and_barrier(self, tick_clock, wait_clock):
    nc = self.nc
    dummy = mybir.InstNoOp(
        name=f"I-tailwaits-{nc.next_id()}",
        engine=mybir.EngineType.SP,
        ins=[],
        outs=[],
    )
    wait_clock.add_sem_waits(dummy, ScopedClock({None: tick_clock.global_clock}))
    waits = list(dummy.sync_info.on_wait) if dummy.sync_info is not None else []
    for w in waits:
        sem = bass_rust.SemaphoreHandle(w.ant_name, w.id)
        op = _WAIT_OP.get(str(w.wait_mode), "sem-ge")
        nc.sync.nop().wait_op(sem, w.wait_value, op)

    nc.sync.drain()

    nc.all_engine_barrier()
    assert self.sems is not None
    popped = nc._tile_sem_poison_stack.pop()
    assert popped is self._sem_poison
    nc.clear_and_free_semaphores(list(self.sems.allocated().values()))
    nc.all_engine_barrier()


tile.TileContext._drain_and_barrier = _patched_drain_and_barrier


from contextlib import ExitStack

import numpy as np

import concourse.bass as bass
from concourse import mybir

FP = mybir.dt.float32
BF = mybir.dt.bfloat16

EMBED_DIM = 192
NUM_HEADS = 4
HEAD_DIM = EMBED_DIM // NUM_HEADS  # 48
SCALE = HEAD_DIM ** -0.5


# ---------------------------------------------------------------- host prep

def prep_weights(w_qkv, b_qkv, w_proj, b_proj):
    """Host-side weight preprocessing (shared by all cores).

    Returns dict of numpy arrays:
      wqk [256, 512]: per f-chunk of 128: [h0(48) pad(16) h1(48) pad(16)],
                      chunks = [q01, q23, k01, k23]; row 192 = bias row;
                      rows 193-255 zero (contraction pad). Q part (incl
                      bias) pre-scaled by 1/sqrt(D).
      wv  [256, 192]: [Wv^T; b_v; zeros]
      wp4 [512, 192]: per head h a [128, 192] block: the head's permuted
                      Wp^T rows at (h%2)*64 + [l-slot, dims...], other
                      rows zero.
      bp  [1, 192]  : b_proj
    """
    d = EMBED_DIM
    wq = w_qkv[0:d] * SCALE          # [192,192] rows = q features
    bq = b_qkv[0:d] * SCALE
    wk = w_qkv[d:2 * d]
    bk = b_qkv[d:2 * d]
    wv = w_qkv[2 * d:3 * d]
    bv = b_qkv[2 * d:3 * d]

    def chunk2(w, b, h0, h1):
        # [256, 128] column block: head h0 cols 0-47, zeros 48-63,
        # head h1 cols 64-111, zeros 112-127; row 192 = bias; rest zero.
        blk = np.zeros((256, 128), dtype=np.float32)
        blk[0:d, 0:48] = w[h0 * 48:(h0 + 1) * 48].T
        blk[d, 0:48] = b[h0 * 48:(h0 + 1) * 48]
        blk[0:d, 64:112] = w[h1 * 48:(h1 + 1) * 48].T
        blk[d, 64:112] = b[h1 * 48:(h1 + 1) * 48]
        return blk

    wqk = np.concatenate(
        [chunk2(wq, bq, 0, 1), chunk2(wq, bq, 2, 3),
         chunk2(wk, bk, 0, 1), chunk2(wk, bk, 2, 3)], axis=1)  # [256, 512]

    # v rhs produces the padded V-block layout directly: for head h the
    # output cols h*128 + (h%2)*64 + [0=ones, 1-32=dims 0-31, 33-48=dims
    # 32-47] (rest zero). Row 192 (the x ones-row) carries the ones column
    # and the v bias.
    wv_aug = np.zeros((256, 512), dtype=np.float32)
    for h in range(4):
        off = h * 128 + (h % 2) * 64
        wv_aug[0:d, off + 1:off + 33] = wv.T[:, h * 48:h * 48 + 32]
        wv_aug[0:d, off + 33:off + 49] = wv.T[:, h * 48 + 32:(h + 1) * 48]
        wv_aug[d, off + 1:off + 33] = bv[h * 48:h * 48 + 32]
        wv_aug[d, off + 33:off + 49] = bv[h * 48 + 32:(h + 1) * 48]
        wv_aug[d, off] = 1.0
    # per-head proj rhs, zero-padded to full 128-row contraction. Head h's
    # z rows sit at partitions (h%2)*64 + [0=l-slot, 1-32=dims 0-31,
    # 33-48=dims 32-47] of the (unnormalized) z^T tile of head-group h//2.
    wp4 = np.zeros((4, 128, 192), dtype=np.float32)
    for h in range(4):
        off = (h % 2) * 64
        wp4[h, off + 1:off + 33] = w_proj.T[h * 48:h * 48 + 32]
        wp4[h, off + 33:off + 49] = w_proj.T[h * 48 + 32:(h + 1) * 48]
    # head pair (2g, 2g+1) concatenated -> one 384-col matmul per group
    wp4 = np.concatenate([wp4[0::2], wp4[1::2]], axis=2)  # [2,128,384]
    bp = np.ascontiguousarray(b_proj[None, :])            # [1, 192]
    return {
        "wqk": np.ascontiguousarray(wqk, dtype=np.float32),
        "wv": wv_aug,
        "wp": wp4.reshape(256, 384).copy(),
        "bp": bp.astype(np.float32),
    }


def prep_x_core(x_core):
    """x_core [P, N, d] -> xT [256, P*N]: x^T, ones row 192, zero pad."""
    P, N, d = x_core.shape
    xt = np.zeros((256, P * N), dtype=np.float32)
    xt[0:d] = x_core.reshape(P * N, d).T
    xt[d] = 1.0
    return xt


# ---------------------------------------------------------------- kernel

def build_nc(P_loc=4, N=1024, repeat=1, nonce=77):
    nc = bass.Bass()
    T = P_loc * N
    xT = nc.dram_tensor("xT", [256, T], BF, kind="ExternalInput")
    wqk = nc.dram_tensor("wqk", [256, 512], BF, kind="ExternalInput")
    wv = nc.dram_tensor("wv", [256, 512], BF, kind="ExternalInput")
    wp = nc.dram_tensor("wp", [256, 384], BF, kind="ExternalInput")
    bp = nc.dram_tensor("bp", [1, 192], FP, kind="ExternalInput")
    # The remote executable cache keys on the I/O signature only (not BIR
    # content); this size-varying dummy input forces a distinct cache slot
    # per kernel revision.
    nc.dram_tensor("nonce", [1, nonce], FP, kind="ExternalInput")
    out = nc.dram_tensor("out", [T, 192], FP, kind="ExternalOutput")

    with tile.TileContext(nc) as tc:
        with tc.tile_pool(name="persist", bufs=1) as persist:
            st = _setup(nc, tc, persist, P_loc, N)
            if repeat > 1:
                with tc.For_i(0, repeat, 1):
                    _body(nc, tc, persist, st, xT, wqk, wv, wp, bp, out,
                          P_loc, N)
            else:
                _body(nc, tc, persist, st, xT, wqk, wv, wp, bp, out,
                      P_loc, N)
    return nc


def _split_multi_waits(nc):
    """Post-pass: walrus accepts at most one sync wait per TPB_CTRL
    instruction, but Tile's loop reset/exit blocks pack several. Replace each
    multi-wait instruction's waits with per-wait NoOps inserted before it."""
    for f in nc.m.functions:
        for bb in f.blocks:
            insts = bb.instructions
            if not any(i.sync_info is not None and len(i.sync_info.on_wait) > 1
                       for i in insts):
                continue
            out = []
            for inst in insts:
                si = inst.sync_info
                if si is not None and len(si.on_wait) > 1:
                    for w in list(si.on_wait):
                        out.append(mybir.InstNoOp(
                            name=f"I-splitw-{nc.next_id()}",
                            engine=inst.engine,
                            ins=[],
                            outs=[],
                            sync_info=mybir.SyncInfo(on_wait=[w],
                                                     on_update=[]),
                            bass_nofuse=True,
                        ))
                    inst.sync_info = mybir.SyncInfo(
                        on_wait=[], on_update=list(si.on_update))
                out.append(inst)
            bb.instructions = out


def _setup(nc, tc, persist, P_loc, N):
    """Allocate persistent SBUF state + one-time zero-init of the padded
    regions (loop-invariant: data writes inside the loop never touch the
    zero pads)."""
    T = P_loc * N
    TT = T // 128
    st = {}
    st["wqk_hi"] = persist.tile([128, 512], BF, tag="wqk_hi", name="wqk_hi")
    st["wqk_lo"] = persist.tile([128, 512], BF, tag="wqk_lo", name="wqk_lo")
    st["xT_hi"] = persist.tile([128, T], BF, tag="xT_hi", name="xT_hi")
    st["xT_lo"] = persist.tile([128, T], BF, tag="xT_lo", name="xT_lo")
    st["wv_hi"] = persist.tile([128, 512], BF, tag="wv_hi", name="wv_hi")
    st["wv_lo"] = persist.tile([128, 512], BF, tag="wv_lo", name="wv_lo")
    st["wp4"] = [persist.tile([128, 384], BF, tag=f"wp4_{h}", name=f"wp4_{h}")
                 for h in range(2)]
    st["bp_sb"] = persist.tile([128, 192], FP, tag="bp_sb", name="bp_sb")
    # K^T per head-group (full padded layout: head A rows 0-47, head B
    # rows 64-111, pad rows zero via the wqk chunk padding).
    st["kT"] = [persist.tile([128, T], BF, tag=f"kT{i}", name=f"kT{i}") for i in range(2)]
    # Q^T per head, zero everywhere except that head's rows: lets S use a
    # full [128,128] K stationary (cross-head terms hit zero Q rows).
    st["qT"] = [persist.tile([128, T], BF, tag=f"qT{h}", name=f"qT{h}") for h in range(4)]
    # V blocks per (token-tile, head): [128, 128]; head h's strip sits at
    # cols (h%2)*64 + [0=ones, 1-32=dims 0-31, 33-48=dims 32-47]; all
    # other cols zero. PV then runs full-stationary with both heads
    # accumulating into the full-width zps (each contributes zeros to the
    # other's partitions).
    st["v_sb"] = persist.tile([128, TT, 4, 128], BF, tag="v_sb", name="v_sb")
    st["warm"] = persist.tile([128, 128], BF, tag="warm", name="warm")

    nc.vector.memset(st["warm"], 0.0)
    for h in (0, 1, 2, 3):
        nc.vector.memset(st["qT"][h][:], 0.0)
    return st


def _body(nc, tc, persist, st, xT, wqk, wv, wp, bp, out, P_loc, N):
    T = P_loc * N
    NK = N // 128            # key tiles per (p, hg)

    wqk_hi, wqk_lo = st["wqk_hi"], st["wqk_lo"]
    xT_hi, xT_lo = st["xT_hi"], st["xT_lo"]
    wv_hi, wv_lo = st["wv_hi"], st["wv_lo"]
    wp4, bp_sb = st["wp4"], st["bp_sb"]
    kT, qT, v_sb, warm = st["kT"], st["qT"], st["v_sb"], st["warm"]

    with ExitStack() as ctx:
        pt_pool = ctx.enter_context(tc.tile_pool(name="pt", bufs=2))
        # zsb/rcol live from pair p's drain until proj(p) finishes during
        # pair p+1 -> up to 4 instances of each tag alive.
        sm_pool = ctx.enter_context(tc.tile_pool(name="sm", bufs=4))
        dr_pool = ctx.enter_context(
            tc.tile_pool(name="dr", bufs=2, space="DRAM"))
        ob_pool = ctx.enter_context(tc.tile_pool(name="ob", bufs=8))
        s_pool = ctx.enter_context(
            tc.tile_pool(name="s", bufs=1, space="PSUM"))
        z_pool = ctx.enter_context(
            tc.tile_pool(name="z", bufs=1, space="PSUM"))
        m_pool = ctx.enter_context(
            tc.tile_pool(name="m", bufs=2, space="PSUM"))

        # ---- input DMAs, ordered so pair 0's qkv inputs land first
        def dma_xT(p):
            sl = slice(p * N, (p + 1) * N)
            nc.sync.dma_start(out=xT_hi[:, sl], in_=xT[0:128, sl])
            nc.sync.dma_start(out=xT_lo[:, sl], in_=xT[128:256, sl])

        nc.sync.dma_start(out=wqk_hi, in_=wqk[0:128, :])
        nc.sync.dma_start(out=xT_hi[:, 0:N], in_=xT[0:128, 0:N])
        nc.sync.dma_start(out=wqk_lo, in_=wqk[128:256, :])
        nc.sync.dma_start(out=xT_lo[:, 0:N], in_=xT[128:256, 0:N])
        nc.sync.dma_start(out=wv_hi, in_=wv[0:128, :])
        nc.sync.dma_start(out=wv_lo, in_=wv[128:256, :])
        dma_xT(1)
        for h in range(2):
            nc.sync.dma_start(out=wp4[h],
                              in_=wp[h * 128:(h + 1) * 128, :])
        nc.sync.dma_start(out=bp_sb, in_=bp[:].to_broadcast([128, 192]))
        dma_xT(2)
        dma_xT(3)

        zsb_tiles = {}
        rcol_tiles = {}

        # -------- emission units
        def qkv_unit(p, fc, half):
            """Both [128,512] half-chunks of q/k feature block fc for pair
            p, hi/lo interleaved across two psum slots so consecutive
            matmuls alternate banks. fc in 0..3 = (q01, q23, k01, k23);
            half ignored (kept for the bg slot bookkeeping)."""
            if half == 1:
                return
            c0 = p * N
            c1 = p * N + 512
            ps0 = m_pool.tile([128, 512], FP, tag="m", name="ps_qk0")
            ps1 = m_pool.tile([128, 512], FP, tag="m", name="ps_qk1")
            lhs_hi = wqk_hi[:, fc * 128:(fc + 1) * 128]
            lhs_lo = wqk_lo[:, fc * 128:(fc + 1) * 128]
            nc.tensor.matmul(ps0, lhsT=lhs_hi, rhs=xT_hi[:, c0:c0 + 512],
                             start=True, stop=False)
            nc.tensor.matmul(ps1, lhsT=lhs_hi, rhs=xT_hi[:, c1:c1 + 512],
                             start=True, stop=False)
            nc.tensor.matmul(ps0, lhsT=lhs_lo, rhs=xT_lo[:, c0:c0 + 512],
                             start=False, stop=True)
            nc.tensor.matmul(ps1, lhsT=lhs_lo, rhs=xT_lo[:, c1:c1 + 512],
                             start=False, stop=True)
            for cs, ps in ((c0, ps0), (c1, ps1)):
                if fc >= 2:
                    nc.vector.tensor_copy(kT[fc - 2][:, cs:cs + 512], ps)
                else:
                    nc.vector.tensor_copy(qT[2 * fc][0:48, cs:cs + 512],
                                          ps[0:48, :])
                    nc.vector.tensor_copy(
                        qT[2 * fc + 1][64:112, cs:cs + 512],
                        ps[64:112, :])

        def v_unit(p, tt):
            """V blocks for global token tile p*NK+tt."""
            ci = p * NK + tt
            sl = slice(ci * 128, (ci + 1) * 128)
            ps = m_pool.tile([128, 512], FP, tag="m", name="ps_v")
            nc.tensor.matmul(ps, lhsT=xT_hi[:, sl], rhs=wv_hi[:],
                             start=True, stop=False)
            nc.tensor.matmul(ps, lhsT=xT_lo[:, sl], rhs=wv_lo[:],
                             start=False, stop=True)
            nc.vector.tensor_copy(
                v_sb[:, ci, :, :].rearrange("p h c -> p (h c)"), ps)

        MUL = mybir.AluOpType.mult
        ADD = mybir.AluOpType.add
        ob_tiles = {}

        def proj_unit(p, tt, parts="ab", tail_alt=False):
            """Output proj for global token tile p*NK+tt.

            Per-head partial projections (full [128,128] z stationary x
            per-head zero-padded wp4), then per-token softmax
            normalization as per-partition scalar multiplies fused with
            the head accumulation (STT ops): the token axis is the
            partition axis here, so no column broadcast is needed.

            parts: "a" = hg0 heads into an ob accumulator, "b" = hg1
            heads + bias + output DMA. Splitting lets pair 3's "a" half
            hide inside its own hg1 loop so only "b" remains in the tail.
            """
            sl = slice((p * NK + tt) * 128, (p * NK + tt + 1) * 128)
            lsl = slice(tt * 128, (tt + 1) * 128)
            if "a" in parts:
                zA = zsb_tiles[(p, 0)]
                rA = rcol_tiles[(p, 0)]
                ps1 = m_pool.tile([128, 512], FP, tag="m", name="ps_pj1")
                nc.tensor.matmul(ps1[:, 0:384], lhsT=zA[:, lsl],
                                 rhs=wp4[0][:], start=True, stop=True,
                                 skip_group_check=True)
                ob = ob_pool.tile([128, 192], FP, tag="ob", name="ob")
                ob_tiles[(p, tt)] = ob
                nc.vector.scalar_tensor_tensor(
                    ob, ps1[:, 0:192], rA[:, 0, tt:tt + 1], bp_sb,
                    op0=MUL, op1=ADD)
                nc.vector.scalar_tensor_tensor(
                    ob, ps1[:, 192:384], rA[:, 1, tt:tt + 1], ob,
                    op0=MUL, op1=ADD)
            if "b" in parts:
                zB = zsb_tiles[(p, 1)]
                rB = rcol_tiles[(p, 1)]
                ob = ob_tiles.pop((p, tt))
                if tail_alt:
                    # tail only: the s_pool banks are idle after the last
                    # exp; alternating rings doubles psum depth so the
                    # final proj dribble is STT-limited, not ring-limited.
                    ps2 = s_pool.tile([128, 512], FP,
                                      tag=("sA" if tt % 2 == 0 else "sB"),
                                      name="ps_pj2")
                else:
                    ps2 = m_pool.tile([128, 512], FP, tag="m",
                                      name="ps_pj2")
                nc.tensor.matmul(ps2[:, 0:384], lhsT=zB[:, lsl],
                                 rhs=wp4[1][:], start=True, stop=True,
                                 skip_group_check=True)
                nc.vector.scalar_tensor_tensor(
                    ob, ps2[:, 0:192], rB[:, 0, tt:tt + 1], ob,
                    op0=MUL, op1=ADD)
                nc.vector.scalar_tensor_tensor(
                    ob, ps2[:, 192:384], rB[:, 1, tt:tt + 1], ob,
                    op0=MUL, op1=ADD)
                nc.sync.dma_start(out=out[sl, :], in_=ob)

        # -------- prologue: PE warmup (HAM un-throttles after ~3.4us of
        # sustained activity; burn the input-DMA wait at 1.2GHz on dummy
        # matmuls so the real stream runs warm), then pair 0 qkv.
        ws = s_pool.tile([128, N], FP, tag="sA", name="warm_ps")
        wexp = pt_pool.tile([128, 16], BF, tag="wexp", name="wexp")
        nc.scalar.activation(wexp, warm[:, 0:16],
                             mybir.ActivationFunctionType.Exp)
        for _ in range(28):
            nc.tensor.matmul(ws[:, 0:128], lhsT=warm[:],
                             rhs=warm[:], start=True, stop=True,
                             skip_group_check=True)
        for fc in (0, 2, 1, 3):
            for half in (0, 1):
                qkv_unit(0, fc, half)

        # -------- main per-pair pipeline
        for p in range(P_loc):
            poff = p * N
            # background units woven into this pair's kt-iterations: qkv of
            # the next pair during hg0 (inputs ready), proj of the previous
            # pair during hg1 (its rcol/zsb are ready by then). Pair 3 has
            # no next-pair qkv, so proj(2) moves up into hg0 and its own
            # proj "a" halves hide in hg1; only the "b" halves remain for
            # the tail.
            bg_qkv = []
            bg_proj = []
            if p + 1 < P_loc:
                for fc in (0, 2, 1, 3):
                    for half in (0, 1):
                        bg_qkv.append((p + 1, fc, half))
            if p >= 1:
                for tt in range(NK):
                    bg_proj.append((p - 1, tt))

            def bg_pop(hg, kt):
                if p < P_loc - 1:
                    if hg == 0 and bg_qkv:
                        qkv_unit(*bg_qkv.pop(0))
                    elif hg == 1 and bg_proj:
                        proj_unit(*bg_proj.pop(0))
                    return
                # pair 3: proj(2) in hg0 (from iter 2, when its rcol chain
                # has landed); own "a" halves in hg1 iters 2-7 (doubled on
                # the last two iters).
                if hg == 0:
                    if kt >= 2 and bg_proj:
                        proj_unit(*bg_proj.pop(0))
                else:
                    if kt < 2 and bg_proj:
                        proj_unit(*bg_proj.pop(0))
                    if kt >= 2:
                        proj_unit(p, kt - 2, parts="a")
                    if kt >= 6:
                        proj_unit(p, kt, parts="a")

            for hg in range(2):
                KH = kT[hg]
                zps = z_pool.tile([128, N], FP, tag="z", name="zps")

                def emit_pv(pv):
                    # both heads' full-width blocks accumulate into the
                    # full zps (each contributes zeros to the other's
                    # partitions); moving operand capped at 512 by this
                    # walrus build -> two chunks.
                    if pv is None:
                        return
                    kt_, pA, pB = pv
                    ci_ = p * NK + kt_
                    # consecutive matmuls alternate psum banks (same-bank
                    # back-to-back writes stall ~100ns/mm)
                    for j, pX in ((0, pA), (1, pB)):
                        for q0 in (0, 512):
                            nc.tensor.matmul(
                                zps[:, q0:q0 + 512],
                                lhsT=v_sb[:, ci_, 2 * hg + j, :],
                                rhs=pX[:, q0:q0 + 512],
                                start=(kt_ == 0 and j == 0),
                                stop=(kt_ == NK - 1 and j == 1),
                                skip_group_check=True)

                pend = None
                for kt in range(NK):
                    if hg == 0:
                        v_unit(p, kt)
                    koff = poff + kt * 128
                    sA = s_pool.tile([128, N], FP, tag="sA", name="sA")
                    sB = s_pool.tile([128, N], FP, tag="sB", name="sB")
                    for q0 in (0, 512):
                        nc.tensor.matmul(
                            sA[:, q0:q0 + 512],
                            lhsT=KH[:, koff:koff + 128],
                            rhs=qT[2 * hg][:, poff + q0:poff + q0 + 512],
                            start=True, stop=True)
                        nc.tensor.matmul(
                            sB[:, q0:q0 + 512],
                            lhsT=KH[:, koff:koff + 128],
                            rhs=qT[2 * hg + 1][:, poff + q0:poff + q0 + 512],
                            start=True, stop=True)
                    ptA = pt_pool.tile([128, N], BF, tag="ptA", name="ptA")
                    ptB = pt_pool.tile([128, N], BF, tag="ptB", name="ptB")
                    nc.scalar.activation(
                        ptA, sA, mybir.ActivationFunctionType.Exp)
                    nc.scalar.activation(
                        ptB, sB, mybir.ActivationFunctionType.Exp)
                    # PV deferred one kt so exp(kt+1)'s S matmuls reach the
                    # PE queue before PV(kt): ACT and PE overlap.
                    emit_pv(pend)
                    pend = (kt, ptA, ptB)
                    bg_pop(hg, kt)
                emit_pv(pend)

                # drain: free zps fast via a DVE copy to SBUF (bf16); pull
                # the l rows (partitions 0 / 64) out through DRAM into a
                # token-major [128, 2, NK] column layout and reciprocal
                # them. Normalization happens inside proj_unit.
                zsb = sm_pool.tile([128, N], BF, tag="zsb", name="zsb")
                nc.vector.tensor_copy(zsb, zps)
                zsb_tiles[(p, hg)] = zsb
                rcol = sm_pool.tile([128, 2, NK], FP, tag="rcol",
                                    name="rcol")
                ldr = dr_pool.tile([2, N], BF, tag="ldr", name="ldr")
                nc.sync.dma_start(out=ldr[0:1, :], in_=zsb[0:1, :])
                nc.sync.dma_start(out=ldr[1:2, :], in_=zsb[64:65, :])
                lcol = sm_pool.tile([128, 2, NK], BF, tag="lcol",
                                    name="lcol")
                nc.sync.dma_start(
                    out=lcol[:],
                    in_=ldr[:].rearrange("t (c q) -> q t c", q=128))
                nc.vector.reciprocal(out=rcol, in_=lcol)
                rcol_tiles[(p, hg)] = rcol

            # any background units not yet emitted (don't drop work)
            while bg_qkv:
                qkv_unit(*bg_qkv.pop(0))
            while bg_proj:
                proj_unit(*bg_proj.pop(0))

        # -------- tail: last pair's proj "b" halves
        for tt in range(NK):
            proj_unit(P_loc - 1, tt, parts="b", tail_alt=True)


# ---------------------------------------------------------------- runner

def make_in_maps(x, w_qkv, b_qkv, w_proj, b_proj, n_cores=8, nonce=77):
    import ml_dtypes
    w = prep_weights(np.asarray(w_qkv), np.asarray(b_qkv),
                     np.asarray(w_proj), np.asarray(b_proj))
    for k in ("wqk", "wv", "wp"):
        w[k] = w[k].astype(ml_dtypes.bfloat16)
    x = np.asarray(x)
    in_maps = []
    for c in range(n_cores):
        m = dict(w)
        m["xT"] = prep_x_core(x[c]).astype(ml_dtypes.bfloat16)
        m["nonce"] = np.zeros((1, nonce), dtype=np.float32)
        in_maps.append(m)
    return in_maps


# ------------------------------------------------------------------ entry

NONCE = 174
B = 8

_CACHED = {}


def _get_nc(repeat=1):
    key = repeat
    if key not in _CACHED:
        nc = build_nc(P_loc=4, N=1024, repeat=repeat,
                      nonce=NONCE + (1 if repeat > 1 else 0))
        _split_multi_waits(nc)
        _CACHED[key] = nc
    return _CACHED[key]


def _make_in_maps(inputs):
    return make_in_maps(inputs["x"], inputs["w_qkv"], inputs["b_qkv"],
                        inputs["w_proj"], inputs["b_proj"], n_cores=B,
                        nonce=NONCE)


def kernel(x, w_qkv, b_qkv, w_proj, b_proj):
    from concourse.bass_utils import run_bass_kernel_spmd

    in_maps = make_in_maps(x, w_qkv, b_qkv, w_proj, b_proj, n_cores=B,
                           nonce=NONCE)
    nc = _get_nc()
    res = run_bass_kernel_spmd(nc, in_maps, core_ids=list(range(B)))
    outs = [res.results[c]["out"].reshape(4, 1024, EMBED_DIM)
            for c in range(B)]
    return np.stack(outs, axis=0).astype(np.float32)


# revision 31
# speedup vs baseline: 1.4441x; 1.4441x over previous
"""Trainium2 Bass kernel for nn_Attention_54262616817926.

kernel(x, w_qkv, b_qkv, w_proj, b_proj) -> out [8, 4, 1024, 192] float32.

Sharding: pure data-parallel over batch B=8 across the 8 NeuronCores
(core c computes batch c end-to-end; no collectives). Inputs are
preprocessed host-side (transposed/augmented layouts).

v4 dataflow, tuned against HW microbenchmarks:
  - Every matmul uses a FULL [128, 128] stationary operand: partial
    stationaries (48-row / 64-col) measure ~2.4x slower per 512-col
    stream on TRN2 (fast-weight-load doesn't engage). Per-head
    zero-padded copies of Q / V-blocks / proj weights make the
    cross-head products vanish exactly while keeping the array full.
  - ACT (exp) is then the bottleneck (~1.23us per [128,1024] exp from
    PSUM): all other work is interleaved into the attention kt-loop's
    program order (qkv of pair p+1, proj of pair p-1, V tiles of pair
    p) so the other engines run in ACT's shadow.
  - Softmax normalization happens inside the output projection: the
    proj output is token-major, so 1/l is a per-partition scalar
    multiply fused into the head accumulation (STT ops on DVE). The l
    rows ride along as a ones-column in the V blocks (partitions 0/64
    of the PV accumulator).
  - One-time zero-init of the padded SBUF regions sits OUTSIDE the
    repeat loop (it is loop-invariant state).

Self-contained: inlines the TileContext tail-drain workaround and the
kernel builder; hardcodes B=8, P=4, N=1024, d=192, H=4.
"""

"""Workaround for walrus 'Too many sync wait commands' on the TileContext
tail drain: this build's walrus accepts at most 1 sync wait on a TPB_CTRL
(Drain) instruction, but TileContext._drain_and_barrier packs every
outstanding semaphore wait onto one drain. Split them into one wait-carrying
nop per semaphore, then emit a clean drain."""

import bass_rust
import concourse.mybir as mybir
import concourse.tile as tile
from concourse.vector_clock import ScopedClock

_WAIT_OP = {
    "ge": "sem-ge",
    "sem-ge": "sem-ge",
}


def _patched_drain_and_barrier(self, tick_clock, wait_clock):
    nc = self.nc
    dummy = mybir.InstNoOp(
        name=f"I-tailwaits-{nc.next_id()}",
        engine=mybir.EngineType.SP,
        ins=[],
        outs=[],
    )
    wait_clock.add_sem_waits(dummy, ScopedClock({None: tick_clock.global_clock}))
    waits = list(dummy.sync_info.on_wait) if dummy.sync_info is not None else []
    for w in waits:
        sem = bass_rust.SemaphoreHandle(w.ant_name, w.id)
        op = _WAIT_OP.get(str(w.wait_mode), "sem-ge")
        nc.sync.nop().wait_op(sem, w.wait_value, op)

    nc.sync.drain()

    nc.all_engine_barrier()
    assert self.sems is not None
    popped = nc._tile_sem_poison_stack.pop()
    assert popped is self._sem_poison
    nc.clear_and_free_semaphores(list(self.sems.allocated().values()))
    nc.all_engine_barrier()


tile.TileContext._drain_and_barrier = _patched_drain_and_barrier


from contextlib import ExitStack

import numpy as np

import concourse.bass as bass
from concourse import mybir

FP = mybir.dt.float32
BF = mybir.dt.bfloat16

EMBED_DIM = 192
NUM_HEADS = 4
HEAD_DIM = EMBED_DIM // NUM_HEADS  # 48
SCALE = HEAD_DIM ** -0.5


# ---------------------------------------------------------------- host prep

def prep_weights(w_qkv, b_qkv, w_proj, b_proj):
    """Host-side weight preprocessing (shared by all cores).

    Returns dict of numpy arrays:
      wqk [256, 512]: per f-chunk of 128: [h0(48) pad(16) h1(48) pad(16)],
                      chunks = [q01, q23, k01, k23]; row 192 = bias row;
                      rows 193-255 zero (contraction pad). Q part (incl
                      bias) pre-scaled by 1/sqrt(D).
      wv  [256, 192]: [Wv^T; b_v; zeros]
      wp4 [512, 192]: per head h a [128, 192] block: the head's permuted
                      Wp^T rows at (h%2)*64 + [l-slot, dims...], other
                      rows zero.
      bp  [1, 192]  : b_proj
    """
    d = EMBED_DIM
    wq = w_qkv[0:d] * SCALE          # [192,192] rows = q features
    bq = b_qkv[0:d] * SCALE
    wk = w_qkv[d:2 * d]
    bk = b_qkv[d:2 * d]
    wv = w_qkv[2 * d:3 * d]
    bv = b_qkv[2 * d:3 * d]

    def chunk2(w, b, h0, h1):
        # [256, 128] column block: head h0 cols 0-47, zeros 48-63,
        # head h1 cols 64-111, zeros 112-127; row 192 = bias; rest zero.
        blk = np.zeros((256, 128), dtype=np.float32)
        blk[0:d, 0:48] = w[h0 * 48:(h0 + 1) * 48].T
        blk[d, 0:48] = b[h0 * 48:(h0 + 1) * 48]
        blk[0:d, 64:112] = w[h1 * 48:(h1 + 1) * 48].T
        blk[d, 64:112] = b[h1 * 48:(h1 + 1) * 48]
        return blk

    wqk = np.concatenate(
        [chunk2(wq, bq, 0, 1), chunk2(wq, bq, 2, 3),
         chunk2(wk, bk, 0, 1), chunk2(wk, bk, 2, 3)], axis=1)  # [256, 512]

    # v rhs produces the padded V-block layout directly: for head h the
    # output cols h*128 + (h%2)*64 + [0=ones, 1-32=dims 0-31, 33-48=dims
    # 32-47] (rest zero). Row 192 (the x ones-row) carries the ones column
    # and the v bias.
    wv_aug = np.zeros((256, 512), dtype=np.float32)
    for h in range(4):
        off = h * 128 + (h % 2) * 64
        wv_aug[0:d, off + 1:off + 33] = wv.T[:, h * 48:h * 48 + 32]
        wv_aug[0:d, off + 33:off + 49] = wv.T[:, h * 48 + 32:(h + 1) * 48]
        wv_aug[d, off + 1:off + 33] = bv[h * 48:h * 48 + 32]
        wv_aug[d, off + 33:off + 49] = bv[h * 48 + 32:(h + 1) * 48]
        wv_aug[d, off] = 1.0
    # per-head proj rhs, zero-padded to full 128-row contraction. Head h's
    # z rows sit at partitions (h%2)*64 + [0=l-slot, 1-32=dims 0-31,
    # 33-48=dims 32-47] of the (unnormalized) z^T tile of head-group h//2.
    wp4 = np.zeros((4, 128, 192), dtype=np.float32)
    for h in range(4):
        off = (h % 2) * 64
        wp4[h, off + 1:off + 33] = w_proj.T[h * 48:h * 48 + 32]
        wp4[h, off + 33:off + 49] = w_proj.T[h * 48 + 32:(h + 1) * 48]
    # head pair (2g, 2g+1) concatenated -> one 384-col matmul per group
    wp4 = np.concatenate([wp4[0::2], wp4[1::2]], axis=2)  # [2,128,384]
    bp = np.ascontiguousarray(b_proj[None, :])            # [1, 192]
    return {
        "wqk": np.ascontiguousarray(wqk, dtype=np.float32),
        "wv": wv_aug,
        "wp": wp4.reshape(256, 384).copy(),
        "bp": bp.astype(np.float32),
    }


def prep_x_core(x_core):
    """x_core [P, N, d] -> xT [256, P*N]: x^T, ones row 192, zero pad."""
    P, N, d = x_core.shape
    xt = np.zeros((256, P * N), dtype=np.float32)
    xt[0:d] = x_core.reshape(P * N, d).T
    xt[d] = 1.0
    return xt


# ---------------------------------------------------------------- kernel

def build_nc(P_loc=4, N=1024, repeat=1, nonce=77):
    nc = bass.Bass()
    T = P_loc * N
    xT = nc.dram_tensor("xT", [256, T], BF, kind="ExternalInput")
    wqk = nc.dram_tensor("wqk", [256, 512], BF, kind="ExternalInput")
    wv = nc.dram_tensor("wv", [256, 512], BF, kind="ExternalInput")
    wp = nc.dram_tensor("wp", [256, 384], BF, kind="ExternalInput")
    bp = nc.dram_tensor("bp", [1, 192], FP, kind="ExternalInput")
    # The remote executable cache keys on the I/O signature only (not BIR
    # content); this size-varying dummy input forces a distinct cache slot
    # per kernel revision.
    nc.dram_tensor("nonce", [1, nonce], FP, kind="ExternalInput")
    out = nc.dram_tensor("out", [T, 192], FP, kind="ExternalOutput")

    with tile.TileContext(nc) as tc:
        with tc.tile_pool(name="persist", bufs=1) as persist:
            st = _setup(nc, tc, persist, P_loc, N)
            if repeat > 1:
                with tc.For_i(0, repeat, 1):
                    _body(nc, tc, persist, st, xT, wqk, wv, wp, bp, out,
                          P_loc, N)
            else:
                _body(nc, tc, persist, st, xT, wqk, wv, wp, bp, out,
                      P_loc, N)
    return nc


def _split_multi_waits(nc):
    """Post-pass: walrus accepts at most one sync wait per TPB_CTRL
    instruction, but Tile's loop reset/exit blocks pack several. Replace each
    multi-wait instruction's waits with per-wait NoOps inserted before it."""
    for f in nc.m.functions:
        for bb in f.blocks:
            insts = bb.instructions
            if not any(i.sync_info is not None and len(i.sync_info.on_wait) > 1
                       for i in insts):
                continue
            out = []
            for inst in insts:
                si = inst.sync_info
                if si is not None and len(si.on_wait) > 1:
                    for w in list(si.on_wait):
                        out.append(mybir.InstNoOp(
                            name=f"I-splitw-{nc.next_id()}",
                            engine=inst.engine,
                            ins=[],
                            outs=[],
                            sync_info=mybir.SyncInfo(on_wait=[w],
                                                     on_update=[]),
                            bass_nofuse=True,
                        ))
                    inst.sync_info = mybir.SyncInfo(
                        on_wait=[], on_update=list(si.on_update))
                out.append(inst)
            bb.instructions = out


def _setup(nc, tc, persist, P_loc, N):
    """Allocate persistent SBUF state + one-time zero-init of the padded
    regions (loop-invariant: data writes inside the loop never touch the
    zero pads)."""
    T = P_loc * N
    TT = T // 128
    st = {}
    st["wqk_hi"] = persist.tile([128, 512], BF, tag="wqk_hi", name="wqk_hi")
    st["wqk_lo"] = persist.tile([128, 512], BF, tag="wqk_lo", name="wqk_lo")
    st["xT_hi"] = persist.tile([128, T], BF, tag="xT_hi", name="xT_hi")
    st["xT_lo"] = persist.tile([128, T], BF, tag="xT_lo", name="xT_lo")
    st["wv_hi"] = persist.tile([128, 512], BF, tag="wv_hi", name="wv_hi")
    st["wv_lo"] = persist.tile([128, 512], BF, tag="wv_lo", name="wv_lo")
    st["wp4"] = [persist.tile([128, 384], BF, tag=f"wp4_{h}", name=f"wp4_{h}")
                 for h in range(2)]
    st["bp_sb"] = persist.tile([128, 192], FP, tag="bp_sb", name="bp_sb")
    # K^T per head-group (full padded layout: head A rows 0-47, head B
    # rows 64-111, pad rows zero via the wqk chunk padding).
    st["kT"] = [persist.tile([128, T], BF, tag=f"kT{i}", name=f"kT{i}") for i in range(2)]
    # Q^T per head, zero everywhere except that head's rows: lets S use a
    # full [128,128] K stationary (cross-head terms hit zero Q rows).
    st["qT"] = [persist.tile([128, T], BF, tag=f"qT{h}", name=f"qT{h}") for h in range(4)]
    # V blocks per (token-tile, head): [128, 128]; head h's strip sits at
    # cols (h%2)*64 + [0=ones, 1-32=dims 0-31, 33-48=dims 32-47]; all
    # other cols zero. PV then runs full-stationary with both heads
    # accumulating into the full-width zps (each contributes zeros to the
    # other's partitions).
    st["v_sb"] = persist.tile([128, TT, 4, 128], BF, tag="v_sb", name="v_sb")
    st["warm"] = persist.tile([128, 128], BF, tag="warm", name="warm")

    nc.vector.memset(st["warm"], 0.0)
    for h in (0, 1, 2, 3):
        nc.vector.memset(st["qT"][h][:], 0.0)
    return st


def _body(nc, tc, persist, st, xT, wqk, wv, wp, bp, out, P_loc, N):
    T = P_loc * N
    NK = N // 128            # key tiles per (p, hg)

    wqk_hi, wqk_lo = st["wqk_hi"], st["wqk_lo"]
    xT_hi, xT_lo = st["xT_hi"], st["xT_lo"]
    wv_hi, wv_lo = st["wv_hi"], st["wv_lo"]
    wp4, bp_sb = st["wp4"], st["bp_sb"]
    kT, qT, v_sb, warm = st["kT"], st["qT"], st["v_sb"], st["warm"]

    with ExitStack() as ctx:
        pt_pool = ctx.enter_context(tc.tile_pool(name="pt", bufs=2))
        # zsb/rcol live from pair p's drain until proj(p) finishes during
        # pair p+1 -> up to 4 instances of each tag alive.
        sm_pool = ctx.enter_context(tc.tile_pool(name="sm", bufs=4))
        dr_pool = ctx.enter_context(
            tc.tile_pool(name="dr", bufs=2, space="DRAM"))
        ob_pool = ctx.enter_context(tc.tile_pool(name="ob", bufs=8))
        s_pool = ctx.enter_context(
            tc.tile_pool(name="s", bufs=1, space="PSUM"))
        z_pool = ctx.enter_context(
            tc.tile_pool(name="z", bufs=1, space="PSUM"))
        m_pool = ctx.enter_context(
            tc.tile_pool(name="m", bufs=2, space="PSUM"))

        # ---- input DMAs, ordered so pair 0's qkv inputs land first
        def dma_xT(p):
            sl = slice(p * N, (p + 1) * N)
            nc.sync.dma_start(out=xT_hi[:, sl], in_=xT[0:128, sl])
            nc.sync.dma_start(out=xT_lo[:, sl], in_=xT[128:256, sl])

        nc.sync.dma_start(out=wqk_hi, in_=wqk[0:128, :])
        nc.sync.dma_start(out=xT_hi[:, 0:N], in_=xT[0:128, 0:N])
        nc.sync.dma_start(out=wqk_lo, in_=wqk[128:256, :])
        nc.sync.dma_start(out=xT_lo[:, 0:N], in_=xT[128:256, 0:N])
        nc.sync.dma_start(out=wv_hi, in_=wv[0:128, :])
        nc.sync.dma_start(out=wv_lo, in_=wv[128:256, :])
        dma_xT(1)
        for h in range(2):
            nc.sync.dma_start(out=wp4[h],
                              in_=wp[h * 128:(h + 1) * 128, :])
        nc.sync.dma_start(out=bp_sb, in_=bp[:].to_broadcast([128, 192]))
        dma_xT(2)
        dma_xT(3)

        zsb_tiles = {}
        rcol_tiles = {}

        # -------- emission units
        def qkv_unit(p, fc, half):
            """One [128,512] chunk of q/k for pair p; fc in 0..3 =
            (q01, q23, k01, k23)."""
            c0 = p * N + half * 512
            ps = m_pool.tile([128, 512], FP, tag="m", name="ps_qk")
            nc.tensor.matmul(ps, lhsT=wqk_hi[:, fc * 128:(fc + 1) * 128],
                             rhs=xT_hi[:, c0:c0 + 512],
                             start=True, stop=False)
            nc.tensor.matmul(ps, lhsT=wqk_lo[:, fc * 128:(fc + 1) * 128],
                             rhs=xT_lo[:, c0:c0 + 512],
                             start=False, stop=True)
            if fc >= 2:
                nc.vector.tensor_copy(kT[fc - 2][:, c0:c0 + 512], ps)
            else:
                nc.vector.tensor_copy(qT[2 * fc][0:48, c0:c0 + 512],
                                      ps[0:48, :])
                nc.vector.tensor_copy(qT[2 * fc + 1][64:112, c0:c0 + 512],
                                      ps[64:112, :])

        def v_unit(p, tt):
            """V blocks for global token tile p*NK+tt."""
            ci = p * NK + tt
            sl = slice(ci * 128, (ci + 1) * 128)
            ps = m_pool.tile([128, 512], FP, tag="m", name="ps_v")
            nc.tensor.matmul(ps, lhsT=xT_hi[:, sl], rhs=wv_hi[:],
                             start=True, stop=False)
            nc.tensor.matmul(ps, lhsT=xT_lo[:, sl], rhs=wv_lo[:],
                             start=False, stop=True)
            nc.vector.tensor_copy(
                v_sb[:, ci, :, :].rearrange("p h c -> p (h c)"), ps)

        MUL = mybir.AluOpType.mult
        ADD = mybir.AluOpType.add
        ob_tiles = {}

        def proj_unit(p, tt, parts="ab", tail_alt=False):
            """Output proj for global token tile p*NK+tt.

            Per-head partial projections (full [128,128] z stationary x
            per-head zero-padded wp4), then per-token softmax
            normalization as per-partition scalar multiplies fused with
            the head accumulation (STT ops): the token axis is the
            partition axis here, so no column broadcast is needed.

            parts: "a" = hg0 heads into an ob accumulator, "b" = hg1
            heads + bias + output DMA. Splitting lets pair 3's "a" half
            hide inside its own hg1 loop so only "b" remains in the tail.
            """
            sl = slice((p * NK + tt) * 128, (p * NK + tt + 1) * 128)
            lsl = slice(tt * 128, (tt + 1) * 128)
            if "a" in parts:
                zA = zsb_tiles[(p, 0)]
                rA = rcol_tiles[(p, 0)]
                ps1 = m_pool.tile([128, 512], FP, tag="m", name="ps_pj1")
                nc.tensor.matmul(ps1[:, 0:384], lhsT=zA[:, lsl],
                                 rhs=wp4[0][:], start=True, stop=True,
                                 skip_group_check=True)
                ob = ob_pool.tile([128, 192], FP, tag="ob", name="ob")
                ob_tiles[(p, tt)] = ob
                nc.vector.scalar_tensor_tensor(
                    ob, ps1[:, 0:192], rA[:, 0, tt:tt + 1], bp_sb,
                    op0=MUL, op1=ADD)
                nc.vector.scalar_tensor_tensor(
                    ob, ps1[:, 192:384], rA[:, 1, tt:tt + 1], ob,
                    op0=MUL, op1=ADD)
            if "b" in parts:
                zB = zsb_tiles[(p, 1)]
                rB = rcol_tiles[(p, 1)]
                ob = ob_tiles.pop((p, tt))
                if tail_alt:
                    # tail only: the s_pool banks are idle after the last
                    # exp; alternating rings doubles psum depth so the
                    # final proj dribble is STT-limited, not ring-limited.
                    ps2 = s_pool.tile([128, 512], FP,
                                      tag=("sA" if tt % 2 == 0 else "sB"),
                                      name="ps_pj2")
                else:
                    ps2 = m_pool.tile([128, 512], FP, tag="m",
                                      name="ps_pj2")
                nc.tensor.matmul(ps2[:, 0:384], lhsT=zB[:, lsl],
                                 rhs=wp4[1][:], start=True, stop=True,
                                 skip_group_check=True)
                nc.vector.scalar_tensor_tensor(
                    ob, ps2[:, 0:192], rB[:, 0, tt:tt + 1], ob,
                    op0=MUL, op1=ADD)
                nc.vector.scalar_tensor_tensor(
                    ob, ps2[:, 192:384], rB[:, 1, tt:tt + 1], ob,
                    op0=MUL, op1=ADD)
                nc.sync.dma_start(out=out[sl, :], in_=ob)

        # -------- prologue: PE warmup (HAM un-throttles after ~3.4us of
        # sustained activity; burn the input-DMA wait at 1.2GHz on dummy
        # matmuls so the real stream runs warm), then pair 0 qkv.
        ws = s_pool.tile([128, N], FP, tag="sA", name="warm_ps")
        wexp = pt_pool.tile([128, 16], BF, tag="wexp", name="wexp")
        nc.scalar.activation(wexp, warm[:, 0:16],
                             mybir.ActivationFunctionType.Exp)
        for _ in range(28):
            nc.tensor.matmul(ws[:, 0:128], lhsT=warm[:],
                             rhs=warm[:], start=True, stop=True,
                             skip_group_check=True)
        for fc in (0, 2, 1, 3):
            for half in (0, 1):
                qkv_unit(0, fc, half)

        # -------- main per-pair pipeline
        for p in range(P_loc):
            poff = p * N
            # background units woven into this pair's kt-iterations: qkv of
            # the next pair during hg0 (inputs ready), proj of the previous
            # pair during hg1 (its rcol/zsb are ready by then). Pair 3 has
            # no next-pair qkv, so proj(2) moves up into hg0 and its own
            # proj "a" halves hide in hg1; only the "b" halves remain for
            # the tail.
            bg_qkv = []
            bg_proj = []
            if p + 1 < P_loc:
                for fc in (0, 2, 1, 3):
                    for half in (0, 1):
                        bg_qkv.append((p + 1, fc, half))
            if p >= 1:
                for tt in range(NK):
                    bg_proj.append((p - 1, tt))

            def bg_pop(hg, kt):
                if p < P_loc - 1:
                    if hg == 0 and bg_qkv:
                        qkv_unit(*bg_qkv.pop(0))
                    elif hg == 1 and bg_proj:
                        proj_unit(*bg_proj.pop(0))
                    return
                # pair 3: proj(2) in hg0 (from iter 2, when its rcol chain
                # has landed); own "a" halves in hg1 iters 2-7 (doubled on
                # the last two iters).
                if hg == 0:
                    if kt >= 2 and bg_proj:
                        proj_unit(*bg_proj.pop(0))
                else:
                    if kt < 2 and bg_proj:
                        proj_unit(*bg_proj.pop(0))
                    if kt >= 2:
                        proj_unit(p, kt - 2, parts="a")
                    if kt >= 6:
                        proj_unit(p, kt, parts="a")

            for hg in range(2):
                KH = kT[hg]
                zps = z_pool.tile([128, N], FP, tag="z", name="zps")

                def emit_pv(pv):
                    # both heads' full-width blocks accumulate into the
                    # full zps (each contributes zeros to the other's
                    # partitions); moving operand capped at 512 by this
                    # walrus build -> two chunks.
                    if pv is None:
                        return
                    kt_, pA, pB = pv
                    ci_ = p * NK + kt_
                    # consecutive matmuls alternate psum banks (same-bank
                    # back-to-back writes stall ~100ns/mm)
                    for j, pX in ((0, pA), (1, pB)):
                        for q0 in (0, 512):
                            nc.tensor.matmul(
                                zps[:, q0:q0 + 512],
                                lhsT=v_sb[:, ci_, 2 * hg + j, :],
                                rhs=pX[:, q0:q0 + 512],
                                start=(kt_ == 0 and j == 0),
                                stop=(kt_ == NK - 1 and j == 1),
                                skip_group_check=True)

                pend = None
                for kt in range(NK):
                    if hg == 0:
                        v_unit(p, kt)
                    koff = poff + kt * 128
                    sA = s_pool.tile([128, N], FP, tag="sA", name="sA")
                    sB = s_pool.tile([128, N], FP, tag="sB", name="sB")
                    for q0 in (0, 512):
                        nc.tensor.matmul(
                            sA[:, q0:q0 + 512],
                            lhsT=KH[:, koff:koff + 128],
                            rhs=qT[2 * hg][:, poff + q0:poff + q0 + 512],
                            start=True, stop=True)
                        nc.tensor.matmul(
                            sB[:, q0:q0 + 512],
                            lhsT=KH[:, koff:koff + 128],
                            rhs=qT[2 * hg + 1][:, poff + q0:poff + q0 + 512],
                            start=True, stop=True)
                    ptA = pt_pool.tile([128, N], BF, tag="ptA", name="ptA")
                    ptB = pt_pool.tile([128, N], BF, tag="ptB", name="ptB")
                    nc.scalar.activation(
                        ptA, sA, mybir.ActivationFunctionType.Exp)
                    nc.scalar.activation(
                        ptB, sB, mybir.ActivationFunctionType.Exp)
                    # PV deferred one kt so exp(kt+1)'s S matmuls reach the
                    # PE queue before PV(kt): ACT and PE overlap.
                    emit_pv(pend)
                    pend = (kt, ptA, ptB)
                    bg_pop(hg, kt)
                emit_pv(pend)

                # drain: free zps fast via a DVE copy to SBUF (bf16); pull
                # the l rows (partitions 0 / 64) out through DRAM into a
                # token-major [128, 2, NK] column layout and reciprocal
                # them. Normalization happens inside proj_unit.
                zsb = sm_pool.tile([128, N], BF, tag="zsb", name="zsb")
                nc.vector.tensor_copy(zsb, zps)
                zsb_tiles[(p, hg)] = zsb
                rcol = sm_pool.tile([128, 2, NK], FP, tag="rcol",
                                    name="rcol")
                ldr = dr_pool.tile([2, N], BF, tag="ldr", name="ldr")
                nc.sync.dma_start(out=ldr[0:1, :], in_=zsb[0:1, :])
                nc.sync.dma_start(out=ldr[1:2, :], in_=zsb[64:65, :])
                lcol = sm_pool.tile([128, 2, NK], BF, tag="lcol",
                                    name="lcol")
                nc.sync.dma_start(
                    out=lcol[:],
                    in_=ldr[:].rearrange("t (c q) -> q t c", q=128))
                nc.vector.reciprocal(out=rcol, in_=lcol)
                rcol_tiles[(p, hg)] = rcol

            # any background units not yet emitted (don't drop work)
            while bg_qkv:
                qkv_unit(*bg_qkv.pop(0))
            while bg_proj:
                proj_unit(*bg_proj.pop(0))

        # -------- tail: last pair's proj "b" halves
        for tt in range(NK):
            proj_unit(P_loc - 1, tt, parts="b", tail_alt=True)


# ---------------------------------------------------------------- runner

def make_in_maps(x, w_qkv, b_qkv, w_proj, b_proj, n_cores=8, nonce=77):
    import ml_dtypes
    w = prep_weights(np.asarray(w_qkv), np.asarray(b_qkv),
                     np.asarray(w_proj), np.asarray(b_proj))
    for k in ("wqk", "wv", "wp"):
        w[k] = w[k].astype(ml_dtypes.bfloat16)
    x = np.asarray(x)
    in_maps = []
    for c in range(n_cores):
        m = dict(w)
        m["xT"] = prep_x_core(x[c]).astype(ml_dtypes.bfloat16)
        m["nonce"] = np.zeros((1, nonce), dtype=np.float32)
        in_maps.append(m)
    return in_maps


# ------------------------------------------------------------------ entry

NONCE = 175
B = 8

_CACHED = {}


def _get_nc(repeat=1):
    key = repeat
    if key not in _CACHED:
        nc = build_nc(P_loc=4, N=1024, repeat=repeat,
                      nonce=NONCE + (1 if repeat > 1 else 0))
        _split_multi_waits(nc)
        _CACHED[key] = nc
    return _CACHED[key]


def _make_in_maps(inputs):
    return make_in_maps(inputs["x"], inputs["w_qkv"], inputs["b_qkv"],
                        inputs["w_proj"], inputs["b_proj"], n_cores=B,
                        nonce=NONCE)


def kernel(x, w_qkv, b_qkv, w_proj, b_proj):
    from concourse.bass_utils import run_bass_kernel_spmd

    in_maps = make_in_maps(x, w_qkv, b_qkv, w_proj, b_proj, n_cores=B,
                           nonce=NONCE)
    nc = _get_nc()
    res = run_bass_kernel_spmd(nc, in_maps, core_ids=list(range(B)))
    outs = [res.results[c]["out"].reshape(4, 1024, EMBED_DIM)
            for c in range(B)]
    return np.stack(outs, axis=0).astype(np.float32)


# revision 32
# speedup vs baseline: 2.6924x; 1.8644x over previous
"""Trainium2 Bass kernel for nn_Attention_54262616817926.

kernel(x, w_qkv, b_qkv, w_proj, b_proj) -> out [8, 4, 1024, 192] float32.

Sharding: pure data-parallel over batch B=8 across the 8 NeuronCores
(core c computes batch c end-to-end; no collectives). Inputs are
preprocessed host-side (transposed/augmented layouts).

v4 dataflow, tuned against HW microbenchmarks:
  - Every matmul uses a FULL [128, 128] stationary operand: partial
    stationaries (48-row / 64-col) measure ~2.4x slower per 512-col
    stream on TRN2 (fast-weight-load doesn't engage). Per-head
    zero-padded copies of Q / V-blocks / proj weights make the
    cross-head products vanish exactly while keeping the array full.
  - ACT (exp) is then the bottleneck (~1.23us per [128,1024] exp from
    PSUM): all other work is interleaved into the attention kt-loop's
    program order (qkv of pair p+1, proj of pair p-1, V tiles of pair
    p) so the other engines run in ACT's shadow.
  - Softmax normalization happens inside the output projection: the
    proj output is token-major, so 1/l is a per-partition scalar
    multiply fused into the head accumulation (STT ops on DVE). The l
    rows ride along as a ones-column in the V blocks (partitions 0/64
    of the PV accumulator).
  - One-time zero-init of the padded SBUF regions sits OUTSIDE the
    repeat loop (it is loop-invariant state).

Self-contained: inlines the TileContext tail-drain workaround and the
kernel builder; hardcodes B=8, P=4, N=1024, d=192, H=4.
"""

"""Workaround for walrus 'Too many sync wait commands' on the TileContext
tail drain: this build's walrus accepts at most 1 sync wait on a TPB_CTRL
(Drain) instruction, but TileContext._drain_and_barrier packs every
outstanding semaphore wait onto one drain. Split them into one wait-carrying
nop per semaphore, then emit a clean drain."""

import bass_rust
import concourse.mybir as mybir
import concourse.tile as tile
from concourse.vector_clock import ScopedClock

_WAIT_OP = {
    "ge": "sem-ge",
    "sem-ge": "sem-ge",
}


def _patched_drain_and_barrier(self, tick_clock, wait_clock):
    nc = self.nc
    dummy = mybir.InstNoOp(
        name=f"I-tailwaits-{nc.next_id()}",
        engine=mybir.EngineType.SP,
        ins=[],
        outs=[],
    )
    wait_clock.add_sem_waits(dummy, ScopedClock({None: tick_clock.global_clock}))
    waits = list(dummy.sync_info.on_wait) if dummy.sync_info is not None else []
    for w in waits:
        sem = bass_rust.SemaphoreHandle(w.ant_name, w.id)
        op = _WAIT_OP.get(str(w.wait_mode), "sem-ge")
        nc.sync.nop().wait_op(sem, w.wait_value, op)

    nc.sync.drain()

    nc.all_engine_barrier()
    assert self.sems is not None
    popped = nc._tile_sem_poison_stack.pop()
    assert popped is self._sem_poison
    nc.clear_and_free_semaphores(list(self.sems.allocated().values()))
    nc.all_engine_barrier()


tile.TileContext._drain_and_barrier = _patched_drain_and_barrier


from contextlib import ExitStack

import numpy as np

import concourse.bass as bass
from concourse import mybir

FP = mybir.dt.float32
BF = mybir.dt.bfloat16

EMBED_DIM = 192
NUM_HEADS = 4
HEAD_DIM = EMBED_DIM // NUM_HEADS  # 48
SCALE = HEAD_DIM ** -0.5


# ---------------------------------------------------------------- host prep

def prep_weights(w_qkv, b_qkv, w_proj, b_proj):
    """Host-side weight preprocessing (shared by all cores).

    Returns dict of numpy arrays:
      wqk [256, 512]: per f-chunk of 128: [h0(48) pad(16) h1(48) pad(16)],
                      chunks = [q01, q23, k01, k23]; row 192 = bias row;
                      rows 193-255 zero (contraction pad). Q part (incl
                      bias) pre-scaled by 1/sqrt(D).
      wv  [256, 192]: [Wv^T; b_v; zeros]
      wp4 [512, 192]: per head h a [128, 192] block: the head's permuted
                      Wp^T rows at (h%2)*64 + [l-slot, dims...], other
                      rows zero.
      bp  [1, 192]  : b_proj
    """
    d = EMBED_DIM
    wq = w_qkv[0:d] * SCALE          # [192,192] rows = q features
    bq = b_qkv[0:d] * SCALE
    wk = w_qkv[d:2 * d]
    bk = b_qkv[d:2 * d]
    wv = w_qkv[2 * d:3 * d]
    bv = b_qkv[2 * d:3 * d]

    def chunk2(w, b, h0, h1):
        # [256, 128] column block: head h0 cols 0-47, zeros 48-63,
        # head h1 cols 64-111, zeros 112-127; row 192 = bias; rest zero.
        blk = np.zeros((256, 128), dtype=np.float32)
        blk[0:d, 0:48] = w[h0 * 48:(h0 + 1) * 48].T
        blk[d, 0:48] = b[h0 * 48:(h0 + 1) * 48]
        blk[0:d, 64:112] = w[h1 * 48:(h1 + 1) * 48].T
        blk[d, 64:112] = b[h1 * 48:(h1 + 1) * 48]
        return blk

    wqk = np.concatenate(
        [chunk2(wq, bq, 0, 1), chunk2(wq, bq, 2, 3),
         chunk2(wk, bk, 0, 1), chunk2(wk, bk, 2, 3)], axis=1)  # [256, 512]

    # v rhs produces the padded V-block layout directly: for head h the
    # output cols h*128 + (h%2)*64 + [0=ones, 1-32=dims 0-31, 33-48=dims
    # 32-47] (rest zero). Row 192 (the x ones-row) carries the ones column
    # and the v bias.
    wv_aug = np.zeros((256, 512), dtype=np.float32)
    for h in range(4):
        off = h * 128 + (h % 2) * 64
        wv_aug[0:d, off + 1:off + 33] = wv.T[:, h * 48:h * 48 + 32]
        wv_aug[0:d, off + 33:off + 49] = wv.T[:, h * 48 + 32:(h + 1) * 48]
        wv_aug[d, off + 1:off + 33] = bv[h * 48:h * 48 + 32]
        wv_aug[d, off + 33:off + 49] = bv[h * 48 + 32:(h + 1) * 48]
        wv_aug[d, off] = 1.0
    # per-head proj rhs, zero-padded to full 128-row contraction. Head h's
    # z rows sit at partitions (h%2)*64 + [0=l-slot, 1-32=dims 0-31,
    # 33-48=dims 32-47] of the (unnormalized) z^T tile of head-group h//2.
    wp4 = np.zeros((4, 128, 192), dtype=np.float32)
    for h in range(4):
        off = (h % 2) * 64
        wp4[h, off + 1:off + 33] = w_proj.T[h * 48:h * 48 + 32]
        wp4[h, off + 33:off + 49] = w_proj.T[h * 48 + 32:(h + 1) * 48]
    # head pair (2g, 2g+1) concatenated -> one 384-col matmul per group
    wp4 = np.concatenate([wp4[0::2], wp4[1::2]], axis=2)  # [2,128,384]
    bp = np.ascontiguousarray(b_proj[None, :])            # [1, 192]
    return {
        "wqk": np.ascontiguousarray(wqk, dtype=np.float32),
        "wv": wv_aug,
        "wp": wp4.reshape(256, 384).copy(),
        "bp": bp.astype(np.float32),
    }


def prep_x_core(x_core):
    """x_core [P, N, d] -> xT [256, P*N]: x^T, ones row 192, zero pad."""
    P, N, d = x_core.shape
    xt = np.zeros((256, P * N), dtype=np.float32)
    xt[0:d] = x_core.reshape(P * N, d).T
    xt[d] = 1.0
    return xt


# ---------------------------------------------------------------- kernel

def build_nc(P_loc=4, N=1024, repeat=1, nonce=77):
    nc = bass.Bass()
    T = P_loc * N
    xT = nc.dram_tensor("xT", [256, T], BF, kind="ExternalInput")
    wqk = nc.dram_tensor("wqk", [256, 512], BF, kind="ExternalInput")
    wv = nc.dram_tensor("wv", [256, 512], BF, kind="ExternalInput")
    wp = nc.dram_tensor("wp", [256, 384], BF, kind="ExternalInput")
    bp = nc.dram_tensor("bp", [1, 192], FP, kind="ExternalInput")
    # The remote executable cache keys on the I/O signature only (not BIR
    # content); this size-varying dummy input forces a distinct cache slot
    # per kernel revision.
    nc.dram_tensor("nonce", [1, nonce], FP, kind="ExternalInput")
    out = nc.dram_tensor("out", [T, 192], FP, kind="ExternalOutput")

    with tile.TileContext(nc) as tc:
        with tc.tile_pool(name="persist", bufs=1) as persist:
            st = _setup(nc, tc, persist, P_loc, N)
            if repeat > 1:
                with tc.For_i(0, repeat, 1):
                    _body(nc, tc, persist, st, xT, wqk, wv, wp, bp, out,
                          P_loc, N)
            else:
                _body(nc, tc, persist, st, xT, wqk, wv, wp, bp, out,
                      P_loc, N)
    return nc


def _split_multi_waits(nc):
    """Post-pass: walrus accepts at most one sync wait per TPB_CTRL
    instruction, but Tile's loop reset/exit blocks pack several. Replace each
    multi-wait instruction's waits with per-wait NoOps inserted before it."""
    for f in nc.m.functions:
        for bb in f.blocks:
            insts = bb.instructions
            if not any(i.sync_info is not None and len(i.sync_info.on_wait) > 1
                       for i in insts):
                continue
            out = []
            for inst in insts:
                si = inst.sync_info
                if si is not None and len(si.on_wait) > 1:
                    for w in list(si.on_wait):
                        out.append(mybir.InstNoOp(
                            name=f"I-splitw-{nc.next_id()}",
                            engine=inst.engine,
                            ins=[],
                            outs=[],
                            sync_info=mybir.SyncInfo(on_wait=[w],
                                                     on_update=[]),
                            bass_nofuse=True,
                        ))
                    inst.sync_info = mybir.SyncInfo(
                        on_wait=[], on_update=list(si.on_update))
                out.append(inst)
            bb.instructions = out


def _setup(nc, tc, persist, P_loc, N):
    """Allocate persistent SBUF state + one-time zero-init of the padded
    regions (loop-invariant: data writes inside the loop never touch the
    zero pads)."""
    T = P_loc * N
    TT = T // 128
    st = {}
    st["wqk_hi"] = persist.tile([128, 512], BF, tag="wqk_hi", name="wqk_hi")
    st["wqk_lo"] = persist.tile([128, 512], BF, tag="wqk_lo", name="wqk_lo")
    st["xT_hi"] = persist.tile([128, T], BF, tag="xT_hi", name="xT_hi")
    st["xT_lo"] = persist.tile([128, T], BF, tag="xT_lo", name="xT_lo")
    st["wv_hi"] = persist.tile([128, 512], BF, tag="wv_hi", name="wv_hi")
    st["wv_lo"] = persist.tile([128, 512], BF, tag="wv_lo", name="wv_lo")
    st["wp4"] = [persist.tile([128, 384], BF, tag=f"wp4_{h}", name=f"wp4_{h}")
                 for h in range(2)]
    st["bp_sb"] = persist.tile([128, 192], FP, tag="bp_sb", name="bp_sb")
    # K^T per head-group (full padded layout: head A rows 0-47, head B
    # rows 64-111, pad rows zero via the wqk chunk padding).
    st["kT"] = [persist.tile([128, T], BF, tag=f"kT{i}", name=f"kT{i}") for i in range(2)]
    # Q^T per head, zero everywhere except that head's rows: lets S use a
    # full [128,128] K stationary (cross-head terms hit zero Q rows).
    st["qT"] = [persist.tile([128, T], BF, tag=f"qT{h}", name=f"qT{h}") for h in range(4)]
    # V blocks per (token-tile, head): [128, 128]; head h's strip sits at
    # cols (h%2)*64 + [0=ones, 1-32=dims 0-31, 33-48=dims 32-47]; all
    # other cols zero. PV then runs full-stationary with both heads
    # accumulating into the full-width zps (each contributes zeros to the
    # other's partitions).
    st["v_sb"] = persist.tile([128, TT, 4, 128], BF, tag="v_sb", name="v_sb")
    st["warm"] = persist.tile([128, 128], BF, tag="warm", name="warm")

    nc.vector.memset(st["warm"], 0.0)
    for h in (0, 1, 2, 3):
        nc.vector.memset(st["qT"][h][:], 0.0)
    return st


def _body(nc, tc, persist, st, xT, wqk, wv, wp, bp, out, P_loc, N):
    T = P_loc * N
    NK = N // 128            # key tiles per (p, hg)

    wqk_hi, wqk_lo = st["wqk_hi"], st["wqk_lo"]
    xT_hi, xT_lo = st["xT_hi"], st["xT_lo"]
    wv_hi, wv_lo = st["wv_hi"], st["wv_lo"]
    wp4, bp_sb = st["wp4"], st["bp_sb"]
    kT, qT, v_sb, warm = st["kT"], st["qT"], st["v_sb"], st["warm"]

    with ExitStack() as ctx:
        pt_pool = ctx.enter_context(tc.tile_pool(name="pt", bufs=2))
        # zsb/rcol live from pair p's drain until proj(p) finishes during
        # pair p+1 -> up to 4 instances of each tag alive.
        sm_pool = ctx.enter_context(tc.tile_pool(name="sm", bufs=4))
        dr_pool = ctx.enter_context(
            tc.tile_pool(name="dr", bufs=2, space="DRAM"))
        ob_pool = ctx.enter_context(tc.tile_pool(name="ob", bufs=8))
        s_pool = ctx.enter_context(
            tc.tile_pool(name="s", bufs=1, space="PSUM"))
        z_pool = ctx.enter_context(
            tc.tile_pool(name="z", bufs=1, space="PSUM"))
        m_pool = ctx.enter_context(
            tc.tile_pool(name="m", bufs=2, space="PSUM"))

        # ---- input DMAs, ordered so pair 0's qkv inputs land first
        def dma_xT(p):
            sl = slice(p * N, (p + 1) * N)
            nc.sync.dma_start(out=xT_hi[:, sl], in_=xT[0:128, sl])
            nc.sync.dma_start(out=xT_lo[:, sl], in_=xT[128:256, sl])

        nc.sync.dma_start(out=wqk_hi, in_=wqk[0:128, :])
        nc.sync.dma_start(out=xT_hi[:, 0:N], in_=xT[0:128, 0:N])
        nc.sync.dma_start(out=wqk_lo, in_=wqk[128:256, :])
        nc.sync.dma_start(out=xT_lo[:, 0:N], in_=xT[128:256, 0:N])
        nc.sync.dma_start(out=wv_hi, in_=wv[0:128, :])
        nc.sync.dma_start(out=wv_lo, in_=wv[128:256, :])
        dma_xT(1)
        for h in range(2):
            nc.sync.dma_start(out=wp4[h],
                              in_=wp[h * 128:(h + 1) * 128, :])
        nc.sync.dma_start(out=bp_sb, in_=bp[:].to_broadcast([128, 192]))
        dma_xT(2)
        dma_xT(3)

        zsb_tiles = {}
        rcol_tiles = {}

        # -------- emission units
        def qkv_unit(p, fc, half):
            """One [128,512] chunk of q/k for pair p; fc in 0..3 =
            (q01, q23, k01, k23)."""
            c0 = p * N + half * 512
            ps = m_pool.tile([128, 512], FP, tag="m", name="ps_qk")
            nc.tensor.matmul(ps, lhsT=wqk_hi[:, fc * 128:(fc + 1) * 128],
                             rhs=xT_hi[:, c0:c0 + 512],
                             start=True, stop=False)
            nc.tensor.matmul(ps, lhsT=wqk_lo[:, fc * 128:(fc + 1) * 128],
                             rhs=xT_lo[:, c0:c0 + 512],
                             start=False, stop=True)
            if fc >= 2:
                nc.vector.tensor_copy(kT[fc - 2][:, c0:c0 + 512], ps)
            else:
                nc.vector.tensor_copy(qT[2 * fc][0:48, c0:c0 + 512],
                                      ps[0:48, :])
                nc.vector.tensor_copy(qT[2 * fc + 1][64:112, c0:c0 + 512],
                                      ps[64:112, :])

        def v_unit(p, tt):
            """V blocks for global token tile p*NK+tt."""
            ci = p * NK + tt
            sl = slice(ci * 128, (ci + 1) * 128)
            ps = m_pool.tile([128, 512], FP, tag="m", name="ps_v")
            nc.tensor.matmul(ps, lhsT=xT_hi[:, sl], rhs=wv_hi[:],
                             start=True, stop=False)
            nc.tensor.matmul(ps, lhsT=xT_lo[:, sl], rhs=wv_lo[:],
                             start=False, stop=True)
            nc.vector.tensor_copy(
                v_sb[:, ci, :, :].rearrange("p h c -> p (h c)"), ps)

        MUL = mybir.AluOpType.mult
        ADD = mybir.AluOpType.add
        ob_tiles = {}

        def proj_unit(p, tt, parts="ab", tail_alt=False):
            """Output proj for global token tile p*NK+tt.

            Per-head partial projections (full [128,128] z stationary x
            per-head zero-padded wp4), then per-token softmax
            normalization as per-partition scalar multiplies fused with
            the head accumulation (STT ops): the token axis is the
            partition axis here, so no column broadcast is needed.

            parts: "a" = hg0 heads into an ob accumulator, "b" = hg1
            heads + bias + output DMA. Splitting lets pair 3's "a" half
            hide inside its own hg1 loop so only "b" remains in the tail.
            """
            sl = slice((p * NK + tt) * 128, (p * NK + tt + 1) * 128)
            lsl = slice(tt * 128, (tt + 1) * 128)
            if "a" in parts:
                zA = zsb_tiles[(p, 0)]
                rA = rcol_tiles[(p, 0)]
                ps1 = m_pool.tile([128, 512], FP, tag="m", name="ps_pj1")
                nc.tensor.matmul(ps1[:, 0:384], lhsT=zA[:, lsl],
                                 rhs=wp4[0][:], start=True, stop=True,
                                 skip_group_check=True)
                ob = ob_pool.tile([128, 192], FP, tag="ob", name="ob")
                ob_tiles[(p, tt)] = ob
                nc.vector.scalar_tensor_tensor(
                    ob, ps1[:, 0:192], rA[:, 0, tt:tt + 1], bp_sb,
                    op0=MUL, op1=ADD)
                nc.vector.scalar_tensor_tensor(
                    ob, ps1[:, 192:384], rA[:, 1, tt:tt + 1], ob,
                    op0=MUL, op1=ADD)
            if "b" in parts:
                zB = zsb_tiles[(p, 1)]
                rB = rcol_tiles[(p, 1)]
                ob = ob_tiles.pop((p, tt))
                if tail_alt:
                    # tail only: the s_pool banks are idle after the last
                    # exp; alternating rings doubles psum depth so the
                    # final proj dribble is STT-limited, not ring-limited.
                    ps2 = s_pool.tile([128, 512], FP,
                                      tag=("sA" if tt % 2 == 0 else "sB"),
                                      name="ps_pj2")
                else:
                    ps2 = m_pool.tile([128, 512], FP, tag="m",
                                      name="ps_pj2")
                nc.tensor.matmul(ps2[:, 0:384], lhsT=zB[:, lsl],
                                 rhs=wp4[1][:], start=True, stop=True,
                                 skip_group_check=True)
                nc.vector.scalar_tensor_tensor(
                    ob, ps2[:, 0:192], rB[:, 0, tt:tt + 1], ob,
                    op0=MUL, op1=ADD)
                nc.vector.scalar_tensor_tensor(
                    ob, ps2[:, 192:384], rB[:, 1, tt:tt + 1], ob,
                    op0=MUL, op1=ADD)
                nc.sync.dma_start(out=out[sl, :], in_=ob)

        # -------- prologue: PE warmup (HAM un-throttles after ~3.4us of
        # sustained activity; burn the input-DMA wait at 1.2GHz on dummy
        # matmuls so the real stream runs warm), then pair 0 qkv.
        ws = s_pool.tile([128, N], FP, tag="sA", name="warm_ps")
        for _ in range(28):
            nc.tensor.matmul(ws[:, 0:128], lhsT=warm[:],
                             rhs=warm[:], start=True, stop=True,
                             skip_group_check=True)
        for fc in (0, 2, 1, 3):
            for half in (0, 1):
                qkv_unit(0, fc, half)

        # -------- main per-pair pipeline
        for p in range(P_loc):
            poff = p * N
            # background units woven into this pair's kt-iterations: qkv of
            # the next pair during hg0 (inputs ready), proj of the previous
            # pair during hg1 (its rcol/zsb are ready by then). Pair 3 has
            # no next-pair qkv, so proj(2) moves up into hg0 and its own
            # proj "a" halves hide in hg1; only the "b" halves remain for
            # the tail.
            bg_qkv = []
            bg_proj = []
            if p + 1 < P_loc:
                for fc in (0, 2, 1, 3):
                    for half in (0, 1):
                        bg_qkv.append((p + 1, fc, half))
            if p >= 1:
                for tt in range(NK):
                    bg_proj.append((p - 1, tt))

            def bg_pop(hg, kt):
                if p < P_loc - 1:
                    if hg == 0 and bg_qkv:
                        qkv_unit(*bg_qkv.pop(0))
                    elif hg == 1 and bg_proj:
                        proj_unit(*bg_proj.pop(0))
                    return
                # pair 3: proj(2) in hg0 (from iter 2, when its rcol chain
                # has landed); own "a" halves in hg1 iters 2-7 (doubled on
                # the last two iters).
                if hg == 0:
                    if kt >= 2 and bg_proj:
                        proj_unit(*bg_proj.pop(0))
                else:
                    if kt < 2 and bg_proj:
                        proj_unit(*bg_proj.pop(0))
                    if kt >= 2:
                        proj_unit(p, kt - 2, parts="a")
                    if kt >= 6:
                        proj_unit(p, kt, parts="a")

            for hg in range(2):
                KH = kT[hg]
                zps = z_pool.tile([128, N], FP, tag="z", name="zps")

                def emit_pv(pv):
                    # both heads' full-width blocks accumulate into the
                    # full zps (each contributes zeros to the other's
                    # partitions); moving operand capped at 512 by this
                    # walrus build -> two chunks.
                    if pv is None:
                        return
                    kt_, pA, pB = pv
                    ci_ = p * NK + kt_
                    # consecutive matmuls alternate psum banks (same-bank
                    # back-to-back writes stall ~100ns/mm)
                    for j, pX in ((0, pA), (1, pB)):
                        for q0 in (0, 512):
                            nc.tensor.matmul(
                                zps[:, q0:q0 + 512],
                                lhsT=v_sb[:, ci_, 2 * hg + j, :],
                                rhs=pX[:, q0:q0 + 512],
                                start=(kt_ == 0 and j == 0),
                                stop=(kt_ == NK - 1 and j == 1),
                                skip_group_check=True)

                pend = None
                for kt in range(NK):
                    if hg == 0:
                        v_unit(p, kt)
                    koff = poff + kt * 128
                    sA = s_pool.tile([128, N], FP, tag="sA", name="sA")
                    sB = s_pool.tile([128, N], FP, tag="sB", name="sB")
                    for q0 in (0, 512):
                        nc.tensor.matmul(
                            sA[:, q0:q0 + 512],
                            lhsT=KH[:, koff:koff + 128],
                            rhs=qT[2 * hg][:, poff + q0:poff + q0 + 512],
                            start=True, stop=True)
                        nc.tensor.matmul(
                            sB[:, q0:q0 + 512],
                            lhsT=KH[:, koff:koff + 128],
                            rhs=qT[2 * hg + 1][:, poff + q0:poff + q0 + 512],
                            start=True, stop=True)
                    ptA = pt_pool.tile([128, N], BF, tag="ptA", name="ptA")
                    ptB = pt_pool.tile([128, N], BF, tag="ptB", name="ptB")
                    nc.scalar.activation(
                        ptA, sA, mybir.ActivationFunctionType.Exp)
                    nc.scalar.activation(
                        ptB, sB, mybir.ActivationFunctionType.Exp)
                    # PV deferred one kt so exp(kt+1)'s S matmuls reach the
                    # PE queue before PV(kt): ACT and PE overlap.
                    emit_pv(pend)
                    pend = (kt, ptA, ptB)
                    bg_pop(hg, kt)
                emit_pv(pend)

                # drain: free zps fast via a DVE copy to SBUF (bf16); pull
                # the l rows (partitions 0 / 64) out through DRAM into a
                # token-major [128, 2, NK] column layout and reciprocal
                # them. Normalization happens inside proj_unit.
                zsb = sm_pool.tile([128, N], BF, tag="zsb", name="zsb")
                nc.vector.tensor_copy(zsb, zps)
                zsb_tiles[(p, hg)] = zsb
                rcol = sm_pool.tile([128, 2, NK], FP, tag="rcol",
                                    name="rcol")
                ldr = dr_pool.tile([2, N], BF, tag="ldr", name="ldr")
                nc.sync.dma_start(out=ldr[0:1, :], in_=zsb[0:1, :])
                nc.sync.dma_start(out=ldr[1:2, :], in_=zsb[64:65, :])
                lcol = sm_pool.tile([128, 2, NK], BF, tag="lcol",
                                    name="lcol")
                nc.sync.dma_start(
                    out=lcol[:],
                    in_=ldr[:].rearrange("t (c q) -> q t c", q=128))
                nc.vector.reciprocal(out=rcol, in_=lcol)
                rcol_tiles[(p, hg)] = rcol

            # any background units not yet emitted (don't drop work)
            while bg_qkv:
                qkv_unit(*bg_qkv.pop(0))
            while bg_proj:
                proj_unit(*bg_proj.pop(0))

        # -------- tail: last pair's proj "b" halves
        for tt in range(NK):
            proj_unit(P_loc - 1, tt, parts="b", tail_alt=True)


# ---------------------------------------------------------------- runner

def make_in_maps(x, w_qkv, b_qkv, w_proj, b_proj, n_cores=8, nonce=77):
    import ml_dtypes
    w = prep_weights(np.asarray(w_qkv), np.asarray(b_qkv),
                     np.asarray(w_proj), np.asarray(b_proj))
    for k in ("wqk", "wv", "wp"):
        w[k] = w[k].astype(ml_dtypes.bfloat16)
    x = np.asarray(x)
    in_maps = []
    for c in range(n_cores):
        m = dict(w)
        m["xT"] = prep_x_core(x[c]).astype(ml_dtypes.bfloat16)
        m["nonce"] = np.zeros((1, nonce), dtype=np.float32)
        in_maps.append(m)
    return in_maps


# ------------------------------------------------------------------ entry

NONCE = 176
B = 8

_CACHED = {}


def _get_nc(repeat=1):
    key = repeat
    if key not in _CACHED:
        nc = build_nc(P_loc=4, N=1024, repeat=repeat,
                      nonce=NONCE + (1 if repeat > 1 else 0))
        _split_multi_waits(nc)
        _CACHED[key] = nc
    return _CACHED[key]


def _make_in_maps(inputs):
    return make_in_maps(inputs["x"], inputs["w_qkv"], inputs["b_qkv"],
                        inputs["w_proj"], inputs["b_proj"], n_cores=B,
                        nonce=NONCE)


def kernel(x, w_qkv, b_qkv, w_proj, b_proj):
    from concourse.bass_utils import run_bass_kernel_spmd

    in_maps = make_in_maps(x, w_qkv, b_qkv, w_proj, b_proj, n_cores=B,
                           nonce=NONCE)
    nc = _get_nc()
    res = run_bass_kernel_spmd(nc, in_maps, core_ids=list(range(B)))
    outs = [res.results[c]["out"].reshape(4, 1024, EMBED_DIM)
            for c in range(B)]
    return np.stack(outs, axis=0).astype(np.float32)
